# revision 74
# baseline (speedup 1.0000x reference)
"""MixerHead kernel for 8 trn2 NeuronCores (Bass/Tile, bf16 matmuls).

Math (reference):
  proj[b,h,l,e]  = sum_d x[b,l,d] Wp[h,e,d] + bp[h,e]
  mixed[b,h,f,e] = sum_{l<=f} Wc[h,f,l] proj[b,h,l,e] + bc[h,f]
  out[b,f,j]     = sum_{h,e} mixed[b,h,f,e] Wo[j, h*E+e] + bo[j]

Sharding: core c = (batch b = c//2, head-pair hp = c%2 -> heads {2hp, 2hp+1}).
Each core computes the bias-free linear part for its (batch, 2 heads) and
writes a partial [L, D] output; host sums the two partials per batch and adds
all bias contributions (folded into a single [L, D] matrix analytically).

Device layout chain (every matmul is out = lhsT.T @ rhs, contraction on the
partition dim):
  phase1: proj[l,e]    lhsT = xT[d, l-tile]          rhs = WpT[d, e(512)]
  phase2: mixedT[e,f]  lhsT = proj[l-tile, e-block]  rhs = WcT[l-tile, f-chunk]
          (WcT is pre-masked tril(Wc).T, packed on host; per 512-f chunk the
           sub-diagonal l-tiles are stored 512-f wide and the diagonal 4x4
           l/f-tile block is stored at 256-f granularity so the upper
           triangle is neither stored nor multiplied)
  phase3: part[f,dout] lhsT = mixedT[e-blk, f-tile]  rhs = WoT[e-blk, dout]

All inputs are prefetched at program start (everything fits in SBUF).  DMA
rings drain descriptors in arrival order at ~360GB/s aggregate, and the Tile
framework tracks DMA-write deps at whole-tile granularity, so loads are split
into separate tiles sized to their deadline and issued in deadline order:
x/wp/wo on the sync queue, the packed Wc per-chunk on the otherwise-idle
gpsimd (SWDGE) queue.
"""

import sys

for _p in ("/opt/trn_rl_repo", "/root/.axon_site/_ro/trn_rl_repo"):
    if _p not in sys.path:
        sys.path.append(_p)

import numpy as np

import ml_dtypes

try:  # make trace requests degrade gracefully if the NTFF hook module is absent
    import antenv.axon_hooks  # noqa: F401
except ImportError:
    import types

    import antenv

    _m = types.ModuleType("antenv.axon_hooks")
    _h = {}
    _m.set_axon_ntff_profile_hook = lambda hook: _h.__setitem__("h", hook)
    _m.get_axon_ntff_profile_hook = lambda: _h.get("h")
    sys.modules["antenv.axon_hooks"] = _m
    antenv.axon_hooks = _m

from concourse import bacc, mybir, tile
from concourse.bass_utils import run_bass_kernel_spmd

B, L, D, H, E = 4, 2048, 1024, 4, 256
F32 = mybir.dt.float32
BF16 = mybir.dt.bfloat16

LT = L // 128   # 16 l-tiles per batch
FC = 4          # f-chunks of 512
DT8 = D // 128  # 8 d-tiles

# Packed Wc layout per 512-f chunk c: 4c sub-diagonal l-tiles at 512-f wide,
# then the diagonal 4x4 l/f-tile block at 128-f granularity (f-tile jj needs
# l-tiles 4c..4c+jj -> jj+1 tiles) -> 2048c + 1280 columns.
WC_CHUNK_COLS = [2048 * c + 1280 for c in range(FC)]
WC_OFFS = [sum(WC_CHUNK_COLS[:c]) for c in range(FC)]
WC_PACK_COLS = sum(WC_CHUNK_COLS)  # 17408

# Set by test harness: run with trace and record exec time.
TRACE = False
LAST_EXEC_NS = None

_cache = {}


def _build_program():
    if "nc" in _cache:
        return _cache["nc"]
    nc = bacc.Bacc("TRN2", target_bir_lowering=False, debug=False, num_devices=8)

    xT = nc.dram_tensor("xT", [D, L], BF16, kind="ExternalInput")
    wpT = nc.dram_tensor("wpT", [D, 2 * E], BF16, kind="ExternalInput")
    wc0 = nc.dram_tensor("wc0", [128, WC_PACK_COLS], BF16, kind="ExternalInput")
    wc1 = nc.dram_tensor("wc1", [128, WC_PACK_COLS], BF16, kind="ExternalInput")
    woT = nc.dram_tensor("woT", [2 * E, D], BF16, kind="ExternalInput")
    part = nc.dram_tensor("part", [L, D], BF16, kind="ExternalOutput")
    wc_dram = [wc0, wc1]

    with tile.TileContext(nc) as tc:
        with (
            tc.tile_pool(name="wp", bufs=1) as wp_pool,
            tc.tile_pool(name="wo", bufs=1) as wo_pool,
            tc.tile_pool(name="xt", bufs=1) as x_pool,
            tc.tile_pool(name="wc", bufs=1) as wc_pool,
            tc.tile_pool(name="proj", bufs=1) as proj_pool,
            tc.tile_pool(name="mix", bufs=1) as mix_pool,
            tc.tile_pool(name="outs", bufs=8) as out_pool,
            tc.tile_pool(name="ps1", bufs=1, space="PSUM") as ps1_pool,
            tc.tile_pool(name="ps2", bufs=2, space="PSUM") as ps2_pool,
            tc.tile_pool(name="ps3", bufs=2, space="PSUM") as ps3_pool,
        ):
            # PE warm-up: dummy matmuls with no DMA dependency run during the
            # startup loads so the HAM clock-gate starts opening (0.65 -> 2.4
            # GHz) before the first real matmul.
            warm = wp_pool.tile([128, 512], BF16, tag="warm")
            nc.gpsimd.memset(warm[:], 0.0)
            ps_w = ps3_pool.tile([128, 512], F32, tag="ps3", name="ps_warm")
            for i in range(6):
                nc.tensor.matmul(
                    ps_w[:], warm[:, :128], warm[:], start=(i == 0), stop=(i == 5)
                )

            # ---- prefetch; separate tiles per deadline, issued in deadline
            # order so ring-FIFO arrival matches need time ----
            xt_tiles = {}  # (c) -> list of (tile, d_tiles_per_piece)

            def load_xt(c, pieces, only_g=None, eng=None, gate=None, gate_eng=None):
                eng = eng or nc.sync
                gate_eng = gate_eng or eng
                w = DT8 * 512 // pieces
                if only_g is None:
                    gs = range(pieces)
                    xt_tiles[c] = ([None] * pieces, DT8 // pieces)
                else:
                    gs = [only_g]
                    if c not in xt_tiles:
                        xt_tiles[c] = ([None] * pieces, DT8 // pieces)
                for g in gs:
                    xt_g = x_pool.tile([128, w], BF16, tag=f"xt{c}_{g}")
                    if gate is not None:
                        # WAW gate: the DMA overwrites this, so it cannot be
                        # hoisted before `gate` exists by the scheduler.
                        cp = getattr(gate_eng, "tensor_copy", None) or gate_eng.copy
                        cp(xt_g[:, :16], gate[:, :16])
                    eng.dma_start(
                        xt_g[:].rearrange("p (t l) -> p t l", t=DT8 // pieces),
                        xT[
                            g * (D // pieces) : (g + 1) * (D // pieces),
                            c * 512 : (c + 1) * 512,
                        ].rearrange("(t p) l -> p t l", p=128),
                    )
                    xt_tiles[c][0][g] = xt_g

            def xt_slice(c, d, i):
                tiles, dper = xt_tiles[c]
                t = tiles[d // dper]
                off = (d % dper) * 512 + i * 128
                return t[:, off : off + 128]

            # wp in deadline-sized pieces: d0 | d1-3 | d4-7
            wp_cuts = [(0, 1), (1, 4), (4, 8)]
            wp_tiles = []
            wp = [None] * DT8

            def load_wp(k):
                lo, hi = wp_cuts[k]
                tl = wp_pool.tile([128, (hi - lo) * 512], BF16, tag=f"wp{k}")
                if hi - lo == 1:
                    nc.scalar.dma_start(tl[:], wpT[lo * 128 : hi * 128, :])
                else:
                    nc.scalar.dma_start(
                        tl[:].rearrange("p (t e) -> p t e", t=hi - lo),
                        wpT[lo * 128 : hi * 128, :].rearrange(
                            "(t p) e -> p t e", p=128
                        ),
                    )
                for d in range(lo, hi):
                    wp[d] = tl[:, (d - lo) * 512 : (d - lo + 1) * 512]
                wp_tiles.append(tl)

            # Startup-critical loads split across two issue queues (DMA issue
            # costs ~600ns of queue time each): xt0 pieces on sync, wp on
            # scalar.  Only these go out unpaced: the DMA rings round-robin
            # among in-flight instructions, so anything issued concurrently
            # steals bandwidth from the startup-critical pieces.
            load_xt(0, pieces=4)  # piece g holds d-tiles 2g, 2g+1
            load_wp(0)
            load_wp(1)
            load_wp(2)
            xt0_last = xt_tiles[0][0][3]  # last startup-critical transfer
            xt0_g2 = xt_tiles[0][0][2]

            # xt1/wo on the scalar queue, WAW-gated behind the last
            # startup-critical piece (xt0 g3): their descriptors then follow
            # the phase1(0)-critical pieces into the DMA rings instead of
            # round-robin-stealing ring bandwidth from them.  xt2/xt3 are
            # issued from the gpsimd queue gated on chunk 0's proj copies
            # (see phase1).  The gates are true WAW data deps (a tiny copy
            # into the DMA's own destination), which the tile scheduler
            # cannot reorder away — and emission order keeps its DMA-order
            # model consistent with reality so its coalesced semaphore
            # targets don't serialize against late transfers.
            load_xt(1, pieces=1, eng=nc.scalar, gate=xt0_g2)

            # Packed Wc, one tile per (head, chunk) so deps resolve per
            # chunk; chunk 0 on scalar behind xt1, later chunks vector-gated
            # on the previous chunk's proj with the DMA on the idle sync
            # queue (see phase1).
            wc_sb = [[None] * FC for _ in range(2)]

            def load_wc(c, gate=None, eng=None, gate_eng=None):
                eng = eng or nc.gpsimd
                gate_eng = gate_eng or eng
                cp = getattr(gate_eng, "tensor_copy", None) or gate_eng.copy
                for hh in range(2):
                    wt = wc_pool.tile(
                        [128, WC_CHUNK_COLS[c]], BF16, tag=f"wc{hh}_{c}"
                    )
                    if gate is not None:
                        cp(wt[:, :16], gate[:, :16])
                    eng.dma_start(
                        wt[:],
                        wc_dram[hh][:, WC_OFFS[c] : WC_OFFS[c] + WC_CHUNK_COLS[c]],
                    )
                    wc_sb[hh][c] = wt

            load_wc(0, gate=xt0_g2, eng=nc.scalar)

            # wo split by output dc-half (phase3 consumes dc0 first), each
            # half [128, 4eb x 512j]
            wo_h = []
            for dc in range(2):
                wt = wo_pool.tile([128, 4 * 512], BF16, tag=f"wo{dc}")
                nc.scalar.copy(wt[:, :16], xt0_last[:, :16])
                nc.scalar.dma_start(
                    wt[:].rearrange("p (t j) -> p t j", t=4),
                    woT[:, dc * 512 : (dc + 1) * 512].rearrange(
                        "(t p) j -> p t j", p=128
                    ),
                )
                wo_h.append(wt)

            # proj/mix as one tile per chunk (not per l-tile / per eb): the
            # consumers need the whole chunk anyway, and fewer tile tags
            # shrink the program epilogue.
            proj_t = [None] * FC
            mix_t = [None] * FC

            def proj_sl(t, eb):
                return proj_t[t // 4][:, (t % 4) * 512 + eb * 128 :][:, :128]

            def phase1(c):
                ps1 = [
                    ps1_pool.tile([128, 2 * E], F32, tag=f"ps1_{i}", name=f"ps1_{c}_{i}")
                    for i in range(4)
                ]
                for d in range(DT8):
                    for i in range(4):
                        nc.tensor.matmul(
                            ps1[i][:],
                            xt_slice(c, d, i),
                            wp[d],
                            start=(d == 0),
                            stop=(d == DT8 - 1),
                        )
                pt = proj_pool.tile(
                    [128, 4 * 512], BF16, tag=f"proj{c}", name=f"proj_{c}"
                )
                proj_t[c] = pt
                for i in range(4):
                    nc.vector.tensor_copy(pt[:, i * 512 : (i + 1) * 512], ps1[i][:])
                if c + 1 < FC:
                    # pace the next wc chunk load off this chunk's proj:
                    # gate copy on vector (naturally ordered after the proj
                    # copies), DMA on the otherwise-idle sync queue.
                    load_wc(c + 1, gate=pt, eng=nc.sync, gate_eng=nc.vector)
                    if c == 0:
                        load_xt(2, pieces=1, eng=nc.sync, gate=pt, gate_eng=nc.vector)
                        load_xt(3, pieces=1, eng=nc.sync, gate=pt, gate_eng=nc.vector)

            def phase2(c):
                # causal: 4c sub-diagonal l-tiles at 512-f, then the diagonal
                # 4x4 tile block at 256-f granularity (mask pre-applied in the
                # packed Wc).
                dia = 4 * c * 512
                mt = mix_pool.tile(
                    [128, 4 * 512], BF16, tag=f"m{c}", name=f"mix_{c}"
                )
                mix_t[c] = mt
                for hh in range(2):
                    wct = wc_sb[hh][c]
                    for eb in (2 * hh, 2 * hh + 1):
                        ps = ps2_pool.tile(
                            [128, 512], F32, tag="ps2", name=f"ps2_{c}_{eb}"
                        )
                        for t in range(4 * c):
                            nc.tensor.matmul(
                                ps[:],
                                proj_sl(t, eb),
                                wct[:, t * 512 : (t + 1) * 512],
                                start=(t == 0),
                                stop=False,
                            )
                        off = dia
                        for jj in range(4):  # diag f-tile jj: l-tiles 4c..4c+jj
                            for j in range(jj + 1):
                                nc.tensor.matmul(
                                    ps[:, jj * 128 : (jj + 1) * 128],
                                    proj_sl(4 * c + j, eb),
                                    wct[:, off + j * 128 : off + (j + 1) * 128],
                                    start=(c == 0 and j == 0),
                                    stop=(j == jj),
                                )
                            off += (jj + 1) * 128
                        nc.scalar.copy(mt[:, eb * 512 : (eb + 1) * 512], ps[:])

            def phase3(c):
                for fi in range(4):
                    ft = c * 4 + fi
                    for dc in range(2):
                        ps = ps3_pool.tile(
                            [128, 512], F32, tag="ps3", name=f"ps3_{ft}_{dc}"
                        )
                        for eb in range(4):
                            nc.tensor.matmul(
                                ps[:],
                                mix_t[c][:, eb * 512 + fi * 128 :][:, :128],
                                wo_h[dc][:, eb * 512 : (eb + 1) * 512],
                                start=(eb == 0),
                                stop=(eb == 3),
                            )
                        if ft == LT - 1 and dc == 1:
                            # final piece in 256-col halves so the last copy
                            # + DMA after the last matmul is as short as
                            # possible (the program epilogue serializes
                            # behind the last DMA's completion).
                            for q in range(2):
                                ot = out_pool.tile(
                                    [128, 256], BF16, tag="outq", name=f"outq_{q}"
                                )
                                nc.vector.tensor_copy(ot[:], ps[:, q * 256 :][:, :256])
                                nc.sync.dma_start(
                                    part[
                                        ft * 128 : (ft + 1) * 128,
                                        dc * 512 + q * 256 : dc * 512 + (q + 1) * 256,
                                    ],
                                    ot[:],
                                )
                            continue
                        ot = out_pool.tile(
                            [128, 512], BF16, tag="out", name=f"out_{ft}_{dc}"
                        )
                        nc.vector.tensor_copy(ot[:], ps[:])
                        nc.sync.dma_start(
                            part[ft * 128 : (ft + 1) * 128, dc * 512 : (dc + 1) * 512],
                            ot[:],
                        )

            # Software-pipelined emission: phase3 shifted one chunk later so
            # out-writes stay off the cold-start critical path.
            phase1(0)
            phase2(0)
            for c in range(1, FC):
                phase1(c)
                phase3(c - 1)
                phase2(c)
            phase3(FC - 1)

    nc.compile()
    _cache["nc"] = nc
    return nc


def _pack_wc_head(wc_h: np.ndarray) -> np.ndarray:
    """tril(Wc[h]) -> [128, WC_PACK_COLS] (bf16): per 512-f chunk c, the 4c
    sub-diagonal l-tiles of WcT = tril(Wc).T at 512-f width, then the
    diagonal 4x4 l/f-tile block at 128-f granularity."""
    m = np.tril(wc_h)  # [f, l]
    blocks = []
    for c in range(FC):
        sub = m[c * 512 : (c + 1) * 512, :]  # [512 f, L l]
        if c > 0:
            full = sub[:, : 4 * c * 128].T.reshape(4 * c, 128, 512)
            blocks.append(full.transpose(1, 0, 2).reshape(128, 4 * c * 512))
        for jj in range(4):  # diag f-tile jj: l-tiles 4c..4c+jj
            q = sub[
                jj * 128 : (jj + 1) * 128, 4 * c * 128 : (4 * c + jj + 1) * 128
            ].T.reshape(jj + 1, 128, 128)
            blocks.append(q.transpose(1, 0, 2).reshape(128, (jj + 1) * 128))
    out = np.concatenate(blocks, axis=1)
    assert out.shape == (128, WC_PACK_COLS)
    return np.ascontiguousarray(out).astype(ml_dtypes.bfloat16)


def kernel(x, Wp, bp, Wc, bc, Wo, bo):
    global LAST_EXEC_NS
    x = np.asarray(x, dtype=np.float32)
    Wp = np.asarray(Wp, dtype=np.float32)
    bp = np.asarray(bp, dtype=np.float32)
    Wc = np.asarray(Wc, dtype=np.float32)
    bc = np.asarray(bc, dtype=np.float32)
    Wo = np.asarray(Wo, dtype=np.float32)
    bo = np.asarray(bo, dtype=np.float32)

    nc = _build_program()

    WoT = np.ascontiguousarray(Wo.T)  # [din, dout]
    wc_packed = [_pack_wc_head(Wc[h]) for h in range(H)]
    wpT_pair = []
    woT_pair = []
    for hp in range(2):
        h0, h1 = 2 * hp, 2 * hp + 1
        wpT_pair.append(
            np.ascontiguousarray(
                np.concatenate([Wp[h0].T, Wp[h1].T], axis=1)
            ).astype(ml_dtypes.bfloat16)
        )
        woT_pair.append(
            np.ascontiguousarray(
                np.concatenate(
                    [WoT[h0 * E : (h0 + 1) * E], WoT[h1 * E : (h1 + 1) * E]], axis=0
                )
            ).astype(ml_dtypes.bfloat16)
        )

    in_maps = []
    for c in range(8):
        b, hp = c // 2, c % 2
        in_maps.append(
            {
                "xT": np.ascontiguousarray(x[b].T).astype(ml_dtypes.bfloat16),
                "wpT": wpT_pair[hp],
                "wc0": wc_packed[2 * hp],
                "wc1": wc_packed[2 * hp + 1],
                "woT": woT_pair[hp],
            }
        )

    res = run_bass_kernel_spmd(
        nc, in_maps, core_ids=list(range(8)), trace=TRACE
    )
    LAST_EXEC_NS = res.exec_time_ns

    # Host: fold all bias terms into one [L, D] matrix.
    # mixed bias = tril-rowsum(Wc)[h,f] * bp[h,e] + bc[h,f]; through Wo:
    rs = np.tril(Wc).sum(axis=2)  # [H, L]
    Wo_hE = Wo.reshape(D, H, E)
    V = np.einsum("he,jhe->hj", bp, Wo_hE)  # [H, D]
    WoSum = Wo_hE.sum(axis=2)  # [D, H]
    bias_total = rs.T @ V + bc.T @ WoSum.T + bo[None, :]  # [L, D]

    out = np.empty((B, L, D), dtype=np.float32)
    for b in range(B):
        out[b] = (
            res.results[2 * b]["part"].astype(np.float32)
            + res.results[2 * b + 1]["part"].astype(np.float32)
            + bias_total
        )
    return out


# revision 75
# speedup vs baseline: 1.1452x; 1.1452x over previous
"""MixerHead kernel for 8 trn2 NeuronCores (Bass/Tile, bf16 matmuls).

Math (reference):
  proj[b,h,l,e]  = sum_d x[b,l,d] Wp[h,e,d] + bp[h,e]
  mixed[b,h,f,e] = sum_{l<=f} Wc[h,f,l] proj[b,h,l,e] + bc[h,f]
  out[b,f,j]     = sum_{h,e} mixed[b,h,f,e] Wo[j, h*E+e] + bo[j]

Sharding: core c = (batch b = c//2, head-pair hp = c%2 -> heads {2hp, 2hp+1}).
Each core computes the bias-free linear part for its (batch, 2 heads) and
writes a partial [L, D] output; host sums the two partials per batch and adds
all bias contributions (folded into a single [L, D] matrix analytically).

Device layout chain (every matmul is out = lhsT.T @ rhs, contraction on the
partition dim):
  phase1: proj[l,e]    lhsT = xT[d, l-tile]          rhs = WpT[d, e(512)]
  phase2: mixedT[e,f]  lhsT = proj[l-tile, e-block]  rhs = WcT[l-tile, f-chunk]
          (WcT is pre-masked tril(Wc).T, packed on host; per 512-f chunk the
           sub-diagonal l-tiles are stored 512-f wide and the diagonal 4x4
           l/f-tile block is stored at 256-f granularity so the upper
           triangle is neither stored nor multiplied)
  phase3: part[f,dout] lhsT = mixedT[e-blk, f-tile]  rhs = WoT[e-blk, dout]

All inputs are prefetched at program start (everything fits in SBUF).  DMA
rings drain descriptors in arrival order at ~360GB/s aggregate, and the Tile
framework tracks DMA-write deps at whole-tile granularity, so loads are split
into separate tiles sized to their deadline and issued in deadline order:
x/wp/wo on the sync queue, the packed Wc per-chunk on the otherwise-idle
gpsimd (SWDGE) queue.
"""

import sys

for _p in ("/opt/trn_rl_repo", "/root/.axon_site/_ro/trn_rl_repo"):
    if _p not in sys.path:
        sys.path.append(_p)

import numpy as np

import ml_dtypes

try:  # make trace requests degrade gracefully if the NTFF hook module is absent
    import antenv.axon_hooks  # noqa: F401
except ImportError:
    import types

    import antenv

    _m = types.ModuleType("antenv.axon_hooks")
    _h = {}
    _m.set_axon_ntff_profile_hook = lambda hook: _h.__setitem__("h", hook)
    _m.get_axon_ntff_profile_hook = lambda: _h.get("h")
    sys.modules["antenv.axon_hooks"] = _m
    antenv.axon_hooks = _m

from concourse import bacc, mybir, tile
from concourse.bass_utils import run_bass_kernel_spmd

B, L, D, H, E = 4, 2048, 1024, 4, 256
F32 = mybir.dt.float32
BF16 = mybir.dt.bfloat16

LT = L // 128   # 16 l-tiles per batch
FC = 4          # f-chunks of 512
DT8 = D // 128  # 8 d-tiles

# Packed Wc layout per 512-f chunk c: 4c sub-diagonal l-tiles at 512-f wide,
# then the diagonal 4x4 l/f-tile block at 128-f granularity (f-tile jj needs
# l-tiles 4c..4c+jj -> jj+1 tiles) -> 2048c + 1280 columns.
WC_CHUNK_COLS = [2048 * c + 1280 for c in range(FC)]
WC_OFFS = [sum(WC_CHUNK_COLS[:c]) for c in range(FC)]
WC_PACK_COLS = sum(WC_CHUNK_COLS)  # 17408

# Set by test harness: run with trace and record exec time.
TRACE = False
LAST_EXEC_NS = None

_cache = {}


def _build_program():
    if "nc" in _cache:
        return _cache["nc"]
    nc = bacc.Bacc("TRN2", target_bir_lowering=False, debug=False, num_devices=8)

    xT = nc.dram_tensor("xT", [D, L], BF16, kind="ExternalInput")
    wpT = nc.dram_tensor("wpT", [D, 2 * E], BF16, kind="ExternalInput")
    wc0 = nc.dram_tensor("wc0", [128, WC_PACK_COLS], BF16, kind="ExternalInput")
    wc1 = nc.dram_tensor("wc1", [128, WC_PACK_COLS], BF16, kind="ExternalInput")
    woT = nc.dram_tensor("woT", [2 * E, D], BF16, kind="ExternalInput")
    part = nc.dram_tensor("part", [L, D], BF16, kind="ExternalOutput")
    wc_dram = [wc0, wc1]

    with tile.TileContext(nc) as tc:
        with (
            tc.tile_pool(name="wp", bufs=1) as wp_pool,
            tc.tile_pool(name="wo", bufs=1) as wo_pool,
            tc.tile_pool(name="xt", bufs=1) as x_pool,
            tc.tile_pool(name="wc", bufs=1) as wc_pool,
            tc.tile_pool(name="proj", bufs=1) as proj_pool,
            tc.tile_pool(name="mix", bufs=1) as mix_pool,
            tc.tile_pool(name="outs", bufs=8) as out_pool,
            tc.tile_pool(name="ps1", bufs=1, space="PSUM") as ps1_pool,
            tc.tile_pool(name="ps2", bufs=2, space="PSUM") as ps2_pool,
            tc.tile_pool(name="ps3", bufs=2, space="PSUM") as ps3_pool,
        ):
            # PE warm-up: dummy matmuls with no DMA dependency run during the
            # startup loads so the HAM clock-gate starts opening (0.65 -> 2.4
            # GHz) before the first real matmul.
            warm = wp_pool.tile([128, 512], BF16, tag="warm")
            nc.gpsimd.memset(warm[:], 0.0)
            ps_w = ps3_pool.tile([128, 512], F32, tag="ps3", name="ps_warm")
            for i in range(6):
                nc.tensor.matmul(
                    ps_w[:], warm[:, :128], warm[:], start=(i == 0), stop=(i == 5)
                )

            # ---- prefetch; separate tiles per deadline, issued in deadline
            # order so ring-FIFO arrival matches need time ----
            xt_tiles = {}  # (c) -> list of (tile, d_tiles_per_piece)

            def load_xt(c, pieces, only_g=None, eng=None, gate=None, gate_eng=None):
                eng = eng or nc.sync
                gate_eng = gate_eng or eng
                w = DT8 * 512 // pieces
                if only_g is None:
                    gs = range(pieces)
                    xt_tiles[c] = ([None] * pieces, DT8 // pieces)
                else:
                    gs = [only_g]
                    if c not in xt_tiles:
                        xt_tiles[c] = ([None] * pieces, DT8 // pieces)
                for g in gs:
                    xt_g = x_pool.tile([128, w], BF16, tag=f"xt{c}_{g}")
                    if gate is not None:
                        # WAW gate: the DMA overwrites this, so it cannot be
                        # hoisted before `gate` exists by the scheduler.
                        cp = getattr(gate_eng, "tensor_copy", None) or gate_eng.copy
                        cp(xt_g[:, :16], gate[:, :16])
                    eng.dma_start(
                        xt_g[:].rearrange("p (t l) -> p t l", t=DT8 // pieces),
                        xT[
                            g * (D // pieces) : (g + 1) * (D // pieces),
                            c * 512 : (c + 1) * 512,
                        ].rearrange("(t p) l -> p t l", p=128),
                    )
                    xt_tiles[c][0][g] = xt_g

            def xt_slice(c, d, i):
                tiles, dper = xt_tiles[c]
                t = tiles[d // dper]
                off = (d % dper) * 512 + i * 128
                return t[:, off : off + 128]

            # wp in deadline-sized pieces: d0 | d1-3 | d4-7
            wp_cuts = [(0, 1), (1, 4), (4, 8)]
            wp_tiles = []
            wp = [None] * DT8

            def load_wp(k):
                lo, hi = wp_cuts[k]
                tl = wp_pool.tile([128, (hi - lo) * 512], BF16, tag=f"wp{k}")
                if hi - lo == 1:
                    nc.scalar.dma_start(tl[:], wpT[lo * 128 : hi * 128, :])
                else:
                    nc.scalar.dma_start(
                        tl[:].rearrange("p (t e) -> p t e", t=hi - lo),
                        wpT[lo * 128 : hi * 128, :].rearrange(
                            "(t p) e -> p t e", p=128
                        ),
                    )
                for d in range(lo, hi):
                    wp[d] = tl[:, (d - lo) * 512 : (d - lo + 1) * 512]
                wp_tiles.append(tl)

            # Startup-critical loads split across two issue queues (DMA issue
            # costs ~600ns of queue time each): xt0 pieces on sync, wp on
            # scalar.  Only these go out unpaced: the DMA rings round-robin
            # among in-flight instructions, so anything issued concurrently
            # steals bandwidth from the startup-critical pieces.
            load_xt(0, pieces=4)  # piece g holds d-tiles 2g, 2g+1
            load_wp(0)
            load_wp(1)
            load_wp(2)
            xt0_last = xt_tiles[0][0][3]  # last startup-critical transfer
            xt0_g2 = xt_tiles[0][0][2]

            # xt1/wo on the scalar queue, WAW-gated behind the last
            # startup-critical piece (xt0 g3): their descriptors then follow
            # the phase1(0)-critical pieces into the DMA rings instead of
            # round-robin-stealing ring bandwidth from them.  xt2/xt3 are
            # issued from the gpsimd queue gated on chunk 0's proj copies
            # (see phase1).  The gates are true WAW data deps (a tiny copy
            # into the DMA's own destination), which the tile scheduler
            # cannot reorder away — and emission order keeps its DMA-order
            # model consistent with reality so its coalesced semaphore
            # targets don't serialize against late transfers.
            load_xt(1, pieces=1, eng=nc.scalar, gate=xt0_g2)

            # Packed Wc, one tile per (head, chunk) so deps resolve per
            # chunk; chunk 0 on scalar behind xt1, later chunks vector-gated
            # on the previous chunk's proj with the DMA on the idle sync
            # queue (see phase1).
            wc_sb = [[None] * FC for _ in range(2)]

            def load_wc(c, gate=None, eng=None, gate_eng=None):
                eng = eng or nc.gpsimd
                gate_eng = gate_eng or eng
                cp = getattr(gate_eng, "tensor_copy", None) or gate_eng.copy
                for hh in range(2):
                    wt = wc_pool.tile(
                        [128, WC_CHUNK_COLS[c]], BF16, tag=f"wc{hh}_{c}"
                    )
                    if gate is not None:
                        cp(wt[:, :16], gate[:, :16])
                    eng.dma_start(
                        wt[:],
                        wc_dram[hh][:, WC_OFFS[c] : WC_OFFS[c] + WC_CHUNK_COLS[c]],
                    )
                    wc_sb[hh][c] = wt

            load_wc(0, gate=xt0_g2, eng=nc.scalar)

            # wo split by output dc-half (phase3 consumes dc0 first), each
            # half [128, 4eb x 512j]
            wo_h = []
            for dc in range(2):
                wt = wo_pool.tile([128, 4 * 512], BF16, tag=f"wo{dc}")
                nc.scalar.copy(wt[:, :16], xt0_last[:, :16])
                nc.scalar.dma_start(
                    wt[:].rearrange("p (t j) -> p t j", t=4),
                    woT[:, dc * 512 : (dc + 1) * 512].rearrange(
                        "(t p) j -> p t j", p=128
                    ),
                )
                wo_h.append(wt)

            # proj/mix as one tile per chunk (not per l-tile / per eb): the
            # consumers need the whole chunk anyway, and fewer tile tags
            # shrink the program epilogue.
            proj_t = [None] * FC
            mix_t = [None] * FC

            def proj_sl(t, eb):
                return proj_t[t // 4][:, (t % 4) * 512 + eb * 128 :][:, :128]

            def phase1(c):
                ps1 = [
                    ps1_pool.tile([128, 2 * E], F32, tag=f"ps1_{i}", name=f"ps1_{c}_{i}")
                    for i in range(4)
                ]
                for d in range(DT8):
                    for i in range(4):
                        nc.tensor.matmul(
                            ps1[i][:],
                            xt_slice(c, d, i),
                            wp[d],
                            start=(d == 0),
                            stop=(d == DT8 - 1),
                        )
                pt = proj_pool.tile(
                    [128, 4 * 512], BF16, tag=f"proj{c}", name=f"proj_{c}"
                )
                proj_t[c] = pt
                for i in range(4):
                    nc.vector.tensor_copy(pt[:, i * 512 : (i + 1) * 512], ps1[i][:])
                if c + 1 < FC:
                    # pace the next wc chunk load off this chunk's proj:
                    # gate copy on vector (naturally ordered after the proj
                    # copies), DMA on the otherwise-idle sync queue.
                    load_wc(c + 1, gate=pt, eng=nc.sync, gate_eng=nc.vector)
                    if c == 0:
                        load_xt(2, pieces=1, eng=nc.sync, gate=pt, gate_eng=nc.vector)
                        load_xt(3, pieces=1, eng=nc.sync, gate=pt, gate_eng=nc.vector)

            def phase2(c):
                # causal: 4c sub-diagonal l-tiles at 512-f, then the diagonal
                # 4x4 tile block at 256-f granularity (mask pre-applied in the
                # packed Wc).
                dia = 4 * c * 512
                mt = mix_pool.tile(
                    [128, 4 * 512], BF16, tag=f"m{c}", name=f"mix_{c}"
                )
                mix_t[c] = mt
                for hh in range(2):
                    wct = wc_sb[hh][c]
                    for eb in (2 * hh, 2 * hh + 1):
                        ps = ps2_pool.tile(
                            [128, 512], F32, tag="ps2", name=f"ps2_{c}_{eb}"
                        )
                        for t in range(4 * c):
                            nc.tensor.matmul(
                                ps[:],
                                proj_sl(t, eb),
                                wct[:, t * 512 : (t + 1) * 512],
                                start=(t == 0),
                                stop=False,
                            )
                        off = dia
                        for jj in range(4):  # diag f-tile jj: l-tiles 4c..4c+jj
                            for j in range(jj + 1):
                                nc.tensor.matmul(
                                    ps[:, jj * 128 : (jj + 1) * 128],
                                    proj_sl(4 * c + j, eb),
                                    wct[:, off + j * 128 : off + (j + 1) * 128],
                                    start=(c == 0 and j == 0),
                                    stop=(j == jj),
                                )
                            off += (jj + 1) * 128
                        nc.scalar.copy(mt[:, eb * 512 : (eb + 1) * 512], ps[:])

            def phase3(c):
                for fi in range(4):
                    ft = c * 4 + fi
                    for dc in range(2):
                        ps = ps3_pool.tile(
                            [128, 512], F32, tag="ps3", name=f"ps3_{ft}_{dc}"
                        )
                        for eb in range(4):
                            nc.tensor.matmul(
                                ps[:],
                                mix_t[c][:, eb * 512 + fi * 128 :][:, :128],
                                wo_h[dc][:, eb * 512 : (eb + 1) * 512],
                                start=(eb == 0),
                                stop=(eb == 3),
                            )
                        if ft == LT - 1 and dc == 1:
                            # final piece in 256-col halves so the last copy
                            # + DMA after the last matmul is as short as
                            # possible (the program epilogue serializes
                            # behind the last DMA's completion).
                            for q in range(2):
                                ot = out_pool.tile(
                                    [128, 256], BF16, tag="outq", name=f"outq_{q}"
                                )
                                nc.scalar.copy(ot[:], ps[:, q * 256 :][:, :256])
                                nc.sync.dma_start(
                                    part[
                                        ft * 128 : (ft + 1) * 128,
                                        dc * 512 + q * 256 : dc * 512 + (q + 1) * 256,
                                    ],
                                    ot[:],
                                )
                            continue
                        ot = out_pool.tile(
                            [128, 512], BF16, tag="out", name=f"out_{ft}_{dc}"
                        )
                        nc.scalar.copy(ot[:], ps[:])
                        nc.sync.dma_start(
                            part[ft * 128 : (ft + 1) * 128, dc * 512 : (dc + 1) * 512],
                            ot[:],
                        )

            # Software-pipelined emission: phase3 shifted one chunk later so
            # out-writes stay off the cold-start critical path.
            phase1(0)
            phase2(0)
            for c in range(1, FC):
                phase1(c)
                phase3(c - 1)
                phase2(c)
            phase3(FC - 1)

    nc.compile()
    _cache["nc"] = nc
    return nc


def _pack_wc_head(wc_h: np.ndarray) -> np.ndarray:
    """tril(Wc[h]) -> [128, WC_PACK_COLS] (bf16): per 512-f chunk c, the 4c
    sub-diagonal l-tiles of WcT = tril(Wc).T at 512-f width, then the
    diagonal 4x4 l/f-tile block at 128-f granularity."""
    m = np.tril(wc_h)  # [f, l]
    blocks = []
    for c in range(FC):
        sub = m[c * 512 : (c + 1) * 512, :]  # [512 f, L l]
        if c > 0:
            full = sub[:, : 4 * c * 128].T.reshape(4 * c, 128, 512)
            blocks.append(full.transpose(1, 0, 2).reshape(128, 4 * c * 512))
        for jj in range(4):  # diag f-tile jj: l-tiles 4c..4c+jj
            q = sub[
                jj * 128 : (jj + 1) * 128, 4 * c * 128 : (4 * c + jj + 1) * 128
            ].T.reshape(jj + 1, 128, 128)
            blocks.append(q.transpose(1, 0, 2).reshape(128, (jj + 1) * 128))
    out = np.concatenate(blocks, axis=1)
    assert out.shape == (128, WC_PACK_COLS)
    return np.ascontiguousarray(out).astype(ml_dtypes.bfloat16)


def kernel(x, Wp, bp, Wc, bc, Wo, bo):
    global LAST_EXEC_NS
    x = np.asarray(x, dtype=np.float32)
    Wp = np.asarray(Wp, dtype=np.float32)
    bp = np.asarray(bp, dtype=np.float32)
    Wc = np.asarray(Wc, dtype=np.float32)
    bc = np.asarray(bc, dtype=np.float32)
    Wo = np.asarray(Wo, dtype=np.float32)
    bo = np.asarray(bo, dtype=np.float32)

    nc = _build_program()

    WoT = np.ascontiguousarray(Wo.T)  # [din, dout]
    wc_packed = [_pack_wc_head(Wc[h]) for h in range(H)]
    wpT_pair = []
    woT_pair = []
    for hp in range(2):
        h0, h1 = 2 * hp, 2 * hp + 1
        wpT_pair.append(
            np.ascontiguousarray(
                np.concatenate([Wp[h0].T, Wp[h1].T], axis=1)
            ).astype(ml_dtypes.bfloat16)
        )
        woT_pair.append(
            np.ascontiguousarray(
                np.concatenate(
                    [WoT[h0 * E : (h0 + 1) * E], WoT[h1 * E : (h1 + 1) * E]], axis=0
                )
            ).astype(ml_dtypes.bfloat16)
        )

    in_maps = []
    for c in range(8):
        b, hp = c // 2, c % 2
        in_maps.append(
            {
                "xT": np.ascontiguousarray(x[b].T).astype(ml_dtypes.bfloat16),
                "wpT": wpT_pair[hp],
                "wc0": wc_packed[2 * hp],
                "wc1": wc_packed[2 * hp + 1],
                "woT": woT_pair[hp],
            }
        )

    res = run_bass_kernel_spmd(
        nc, in_maps, core_ids=list(range(8)), trace=TRACE
    )
    LAST_EXEC_NS = res.exec_time_ns

    # Host: fold all bias terms into one [L, D] matrix.
    # mixed bias = tril-rowsum(Wc)[h,f] * bp[h,e] + bc[h,f]; through Wo:
    rs = np.tril(Wc).sum(axis=2)  # [H, L]
    Wo_hE = Wo.reshape(D, H, E)
    V = np.einsum("he,jhe->hj", bp, Wo_hE)  # [H, D]
    WoSum = Wo_hE.sum(axis=2)  # [D, H]
    bias_total = rs.T @ V + bc.T @ WoSum.T + bo[None, :]  # [L, D]

    out = np.empty((B, L, D), dtype=np.float32)
    for b in range(B):
        out[b] = (
            res.results[2 * b]["part"].astype(np.float32)
            + res.results[2 * b + 1]["part"].astype(np.float32)
            + bias_total
        )
    return out
